# revision 5
# baseline (speedup 1.0000x reference)
"""HarmonyGenerator Trainium2 kernel.

Math: the reference's 3x3 conv on [T,1,1,D] degenerates to a 3-tap conv along
the feature axis (only the kernel's middle row touches data).  Conv and the
three linear heads are both linear, so the conv folds into the head weights
(W' = 3-tap correlation of W along K) and the constant context-embedding rows
plus conv bias fold into the output bias.  The device work is one GEMM:

    out[2048, 168] = [melody | lyrics][2048, 50681] @ W'[50681, 168] + bias

Sharding: K (feature) axis split 8 ways, 6400 rows per core (zero padded).
Each core reads 1/8 of x AND 1/8 of W (~28 MB fp16 -> ~69us memory floor at
the ~410 GB/s measured per-core HBM rate) and produces a partial [168, 2048];
partials are summed on the host during the gather/unshard step.

Device schedule per core (PE-rate ~= DMA-rate, "ridge"):
  per k-tile (lhsT = W tile [128k, 168], moving span 512 = ISA max):
    mm psm[t][128,512] += W[:,0:128]T @ xT[:, t*512:(t+1)*512]  (4 t-blocks)
    cb pairs: two concurrent 40-col matmuls per pass via disjoint column
    groups (tile_position (0,0) and (0,64)) -> 2 passes for all 4 t-blocks
  = 6 PE passes of 512 cycles per k-tile (~1.5us warm).

Warmup matmuls keep the PE HAM clock-gate busy (K=8/8, 2.4 GHz) while the
first DMA chunks land; the x stream (2 HWDGE rings, up to 2 MB chunks) then
feeds the PE slightly faster than it consumes, so HAM never re-throttles.
Output is evicted PSUM->SBUF as fp16 (DVE + ACT in parallel) and stored as
three DMAs split across both rings; the host upcasts and adds the bias.
"""

import os
import numpy as np

import concourse.bacc as bacc
import concourse.mybir as mybir
from concourse.tile import TileContext
from concourse.bass_utils import run_bass_kernel_spmd

# Problem shapes (hardcoded per contract)
T = 2048               # steps = length * 128
D_IN = 50937           # 256 ctx + 256 melody/vel + 50425 lyrics
K_GEMM = 50681         # melody(256) + lyrics(50425) features in the GEMM
N_OUT = 168            # 24 chord + 16 beat + 128 mel
N_CORES = 8
K_PER = 6400           # per-core K (8*6400 = 51200 >= 50681, zero padded)
KT = K_PER // 128      # 50 k-tiles per core
TB = 512               # t-block / moving span (ISA max matmul elements)
NTB = T // TB          # 4

_NC = None
LAST_RESULT = None     # BassKernelResults of the most recent run (for test.py)

DTYPE = os.environ.get("HARMONY_DTYPE", "fp16")
WARMUP_MMS = int(os.environ.get("HARMONY_WARMUP", "12"))


def _in_dt():
    return mybir.dt.float16 if DTYPE == "fp16" else mybir.dt.float32r


def _np_in_dt():
    return np.float16 if DTYPE == "fp16" else np.float32


def _build_nc():
    f32 = mybir.dt.float32
    f16 = mybir.dt.float16
    fin = _in_dt()
    nc = bacc.Bacc()
    xt = nc.dram_tensor("xt", [K_PER, T], fin, kind="ExternalInput")
    w = nc.dram_tensor("w", [128, KT * N_OUT], fin, kind="ExternalInput")
    out = nc.dram_tensor("out", [N_OUT, T], f16, kind="ExternalOutput")

    # k-tiles per DMA chunk: small head chunks so the first matmul fires
    # early, 2 MB steady chunks, small tail so the last matmuls aren't
    # gated on a large transfer.
    X_SCHED = [1, 1, 2, 2] + [4] * 10 + [2, 1, 1]
    W_SCHED = [4, 12, 34]
    assert sum(X_SCHED) == KT and sum(W_SCHED) == KT
    with TileContext(nc) as tc:
        with (
            tc.tile_pool(name="wp", bufs=1) as wp,
            tc.tile_pool(name="xp", bufs=8) as xp,
            tc.tile_pool(name="op", bufs=1) as op,
            tc.tile_pool(name="ps", bufs=1, space="PSUM") as ps,
        ):
            # HAM warm-up: the PE clock-gate holds matmuls at 1.2 GHz until
            # ~3.4us of sustained activity.  Burn the DMA-fill window (no real
            # operands on chip yet) on dummy matmuls so real MMs start at
            # 2.4 GHz and the PE is never idle long enough to re-throttle.
            # memset on DVE so the gpsimd queue starts the W DMAs immediately.
            dm = wp.tile([128, 512], fin, tag="warm", name="warmup")
            nc.vector.memset(dm[:], 0.0)
            ps_warm = ps.tile([128, 512], f32, tag="warm_ps", name="ps_warm")
            for _ in range(WARMUP_MMS):
                nc.tensor.matmul(ps_warm[:], dm[:, 0:128], dm[:], start=True, stop=True)

            # W loads on the gpsimd SWDGE ring, leaving both HWDGE rings
            # (sync + scalar) free to alternate x chunks.
            # w_of[kt] -> (tile, col offset of that k-tile's weights)
            w_of = {}
            kt0 = 0
            for wc, n in enumerate(W_SCHED):
                wt = wp.tile([128, n * N_OUT], fin, tag=f"w{wc}", name=f"w{wc}")
                nc.gpsimd.dma_start(wt[:], w[:, kt0 * N_OUT:(kt0 + n) * N_OUT])
                for j in range(n):
                    w_of[kt0 + j] = (wt, j * N_OUT)
                kt0 += n

            # Persistent accumulators: 4 mel banks + 2 shared cb banks.  Each
            # cb bank holds two t-blocks' [40, TB] outputs col-tiled into
            # partitions 0:40 and 64:104 (concurrent matmuls via tile_position).
            psm = [ps.tile([128, TB], f32, tag=f"m{t}", name=f"psm{t}") for t in range(NTB)]
            psc = [ps.tile([128, TB], f32, tag=f"c{p}", name=f"psc{p}") for p in range(NTB // 2)]

            xc0 = 0
            for xc, xn in enumerate(X_SCHED):
                x_tile = xp.tile([128, xn * T], fin, tag="x", name="x_tile")
                ring = nc.sync if xc % 2 == 0 else nc.scalar
                if xn == 1:
                    ring.dma_start(x_tile[:], xt[xc0 * 128:(xc0 + 1) * 128, :])
                else:
                    ring.dma_start(
                        x_tile[:].rearrange("p (a t) -> p a t", a=xn),
                        xt[xc0 * 128:(xc0 + xn) * 128, :].rearrange(
                            "(a p) t -> p a t", p=128
                        ),
                    )
                for a in range(xn):
                    kt = xc0 + a
                    wt, j = w_of[kt]
                    lhs_m = wt[:, j: j + 128]
                    lhs_c = wt[:, j + 128: j + N_OUT]
                    first, last = kt == 0, kt == KT - 1

                    def rhs_of(t):
                        return x_tile[:, a * T + t * TB: a * T + (t + 1) * TB]

                    def cb_pair(p):
                        # two concurrent 40-col matmuls in distinct col groups
                        nc.tensor.matmul(psc[p][0:40, :], lhs_c, rhs_of(2 * p),
                                         start=first, stop=last, tile_position=(0, 0))
                        nc.tensor.matmul(psc[p][64:104, :], lhs_c, rhs_of(2 * p + 1),
                                         start=first, stop=last, tile_position=(0, 64))

                    if not last:
                        # group by stationary operand: 4 mel MMs, then cb pairs
                        for t in range(NTB):
                            nc.tensor.matmul(psm[t][:], lhs_m, rhs_of(t), start=first, stop=last)
                        cb_pair(0)
                        cb_pair(1)
                    else:
                        # final k-tile: finish banks in eviction order so PSUM
                        # evictions and output DMAs overlap the remaining MMs;
                        # end on the cheap-to-evict cb pair.
                        nc.tensor.matmul(psm[0][:], lhs_m, rhs_of(0), start=first, stop=last)
                        nc.tensor.matmul(psm[1][:], lhs_m, rhs_of(1), start=first, stop=last)
                        o_m01 = op.tile([128, 2 * TB], f16, tag="om01", name="o_m01")
                        nc.vector.tensor_copy(o_m01[:, 0:TB], psm[0][:])
                        nc.vector.tensor_copy(o_m01[:, TB:2 * TB], psm[1][:])
                        nc.sync.dma_start(out[0:128, 0:2 * TB], o_m01[:])

                        cb_pair(0)
                        o_c0 = op.tile([104, TB], f16, tag="oc0", name="o_c0")
                        nc.scalar.copy(o_c0[:], psc[0][0:104, :])
                        nc.scalar.dma_start(out[128:N_OUT, 0:TB], o_c0[0:40, :])
                        nc.scalar.dma_start(out[128:N_OUT, TB:2 * TB], o_c0[64:104, :])

                        nc.tensor.matmul(psm[2][:], lhs_m, rhs_of(2), start=first, stop=last)
                        nc.tensor.matmul(psm[3][:], lhs_m, rhs_of(3), start=first, stop=last)
                        o_m23 = op.tile([128, 2 * TB], f16, tag="om23", name="o_m23")
                        nc.vector.tensor_copy(o_m23[:, 0:TB], psm[2][:])
                        nc.vector.tensor_copy(o_m23[:, TB:2 * TB], psm[3][:])
                        nc.sync.dma_start(out[0:128, 2 * TB:T], o_m23[:])

                        cb_pair(1)
                        o_c1 = op.tile([104, TB], f16, tag="oc1", name="o_c1")
                        nc.scalar.copy(o_c1[:], psc[1][0:104, :])
                        nc.scalar.dma_start(out[128:N_OUT, 2 * TB:3 * TB], o_c1[0:40, :])
                        nc.scalar.dma_start(out[128:N_OUT, 3 * TB:T], o_c1[64:104, :])
                xc0 += xn
    return nc


def _get_nc():
    global _NC
    if _NC is None:
        _NC = _build_nc()
        if not _NC.is_finalized():
            _NC.finalize()
    return _NC


def kernel(**inputs):
    global LAST_RESULT
    melody = np.ascontiguousarray(np.asarray(inputs["melody_tensor"], dtype=np.float32))
    lyrics = np.ascontiguousarray(np.asarray(inputs["lyrics_tensor"], dtype=np.float32))
    emb = np.asarray(inputs["emb"], dtype=np.float32)
    conv_w = np.asarray(inputs["conv_w"], dtype=np.float32)
    conv_b = np.asarray(inputs["conv_b"], dtype=np.float32)
    w_chord = np.asarray(inputs["w_chord"], dtype=np.float32)
    w_beat = np.asarray(inputs["w_beat"], dtype=np.float32)
    w_mel = np.asarray(inputs["w_mel"], dtype=np.float32)
    b_heads = np.concatenate([
        np.asarray(inputs["b_chord"], dtype=np.float32),
        np.asarray(inputs["b_beat"], dtype=np.float32),
        np.asarray(inputs["b_mel"], dtype=np.float32),
    ])
    genre = int(np.asarray(inputs["genre"]).reshape(-1)[0])
    tempo = int(np.asarray(inputs["tempo"]).reshape(-1)[0])
    key_sig = int(np.asarray(inputs["key_sig"]).reshape(-1)[0])

    # Fold conv into head weights: W'[e] = k0*W[e+1] + k1*W[e] + k2*W[e-1]
    W = np.concatenate([w_chord, w_beat, w_mel], axis=1)  # [50937, 168]
    k0, k1, k2 = (float(v) for v in conv_w[0, 0, 1, :])
    Wp = k1 * W
    Wp[:-1] += k0 * W[1:]
    Wp[1:] += k2 * W[:-1]

    # Bias: head biases + conv bias * colsum(W) + context-embedding term
    ids = [genre, 10 + tempo, 20 + key_sig, 34]
    ctx = emb[ids].sum(axis=0).astype(np.float64)  # [256]
    bias = (
        b_heads.astype(np.float64)
        + float(conv_b[0]) * W.sum(axis=0, dtype=np.float64)
        + ctx @ Wp[0:256].astype(np.float64)
    )  # [168]

    # Device operands: xT [51200, 2048] (zero padded), W' rows 256.. packed
    np_dt = _np_in_dt()
    K_PAD = N_CORES * K_PER
    XT = np.zeros((K_PAD, T), np_dt)
    XT[0:256] = melody.T
    XT[256:K_GEMM] = lyrics.T
    Wg = np.zeros((K_PAD, N_OUT), np_dt)
    Wg[0:K_GEMM] = Wp[256:]

    in_maps = []
    for c in range(N_CORES):
        wc = (
            Wg[c * K_PER:(c + 1) * K_PER]
            .reshape(KT, 128, N_OUT)
            .transpose(1, 0, 2)
            .reshape(128, KT * N_OUT)
        )
        in_maps.append({
            "xt": XT[c * K_PER:(c + 1) * K_PER],
            "w": np.ascontiguousarray(wc),
        })

    trace = bool(os.environ.get("HARMONY_TRACE"))
    res = run_bass_kernel_spmd(_get_nc(), in_maps, core_ids=list(range(N_CORES)), trace=trace)
    LAST_RESULT = res

    acc = np.zeros((N_OUT, T), np.float64)
    for r in res.results:
        acc += r["out"].astype(np.float64)
    out = (acc + bias[:, None]).T
    return np.ascontiguousarray(out.astype(np.float32))


# revision 7
# speedup vs baseline: 1.0101x; 1.0101x over previous
"""HarmonyGenerator Trainium2 kernel.

Math: the reference's 3x3 conv on [T,1,1,D] degenerates to a 3-tap conv along
the feature axis (only the kernel's middle row touches data).  Conv and the
three linear heads are both linear, so the conv folds into the head weights
(W' = 3-tap correlation of W along K) and the constant context-embedding rows
plus conv bias fold into the output bias.  The device work is one GEMM:

    out[2048, 168] = [melody | lyrics][2048, 50681] @ W'[50681, 168] + bias

Sharding: K (feature) axis split 8 ways, 6400 rows per core (zero padded).
Each core reads 1/8 of x AND 1/8 of W (~28 MB fp16 -> ~69us memory floor at
the ~410 GB/s measured per-core HBM rate) and produces a partial [168, 2048];
partials are summed on the host during the gather/unshard step.

Device schedule per core (PE-rate ~= DMA-rate, "ridge"):
  per k-tile (lhsT = W tile [128k, 168], moving span 512 = ISA max):
    mm psm[t][128,512] += W[:,0:128]T @ xT[:, t*512:(t+1)*512]  (4 t-blocks)
    cb pairs: two concurrent 40-col matmuls per pass via disjoint column
    groups (tile_position (0,0) and (0,64)) -> 2 passes for all 4 t-blocks
  = 6 PE passes of 512 cycles per k-tile (~1.5us warm).

Warmup matmuls keep the PE HAM clock-gate busy (K=8/8, 2.4 GHz) while the
first DMA chunks land; the x stream (2 HWDGE rings, up to 2 MB chunks) then
feeds the PE slightly faster than it consumes, so HAM never re-throttles.
Output is evicted PSUM->SBUF as fp16 (DVE + ACT in parallel) and stored as
three DMAs split across both rings; the host upcasts and adds the bias.
"""

import os
import numpy as np

import concourse.bacc as bacc
import concourse.mybir as mybir
from concourse.tile import TileContext
from concourse.bass_utils import run_bass_kernel_spmd

# Problem shapes (hardcoded per contract)
T = 2048               # steps = length * 128
D_IN = 50937           # 256 ctx + 256 melody/vel + 50425 lyrics
K_GEMM = 50681         # melody(256) + lyrics(50425) features in the GEMM
N_OUT = 168            # 24 chord + 16 beat + 128 mel
N_CORES = 8
K_PER = 6400           # per-core K (8*6400 = 51200 >= 50681, zero padded)
KT = K_PER // 128      # 50 k-tiles per core
TB = 512               # t-block / moving span (ISA max matmul elements)
NTB = T // TB          # 4

_NC = None
LAST_RESULT = None     # BassKernelResults of the most recent run (for test.py)

DTYPE = os.environ.get("HARMONY_DTYPE", "fp16")
WARMUP_MMS = int(os.environ.get("HARMONY_WARMUP", "12"))


def _in_dt():
    return mybir.dt.float16 if DTYPE == "fp16" else mybir.dt.float32r


def _np_in_dt():
    return np.float16 if DTYPE == "fp16" else np.float32


def _build_nc():
    f32 = mybir.dt.float32
    f16 = mybir.dt.float16
    fin = _in_dt()
    nc = bacc.Bacc()
    xt = nc.dram_tensor("xt", [K_PER, T], fin, kind="ExternalInput")
    w = nc.dram_tensor("w", [128, KT * N_OUT], fin, kind="ExternalInput")
    out = nc.dram_tensor("out", [N_OUT, T], f16, kind="ExternalOutput")

    # k-tiles per DMA chunk.  Outstanding DMAs round-robin at packet
    # granularity, so the in-order completion frontier advances only as
    # fast as the aggregate queue drains: keep head chunks small (fast
    # first k-tiles) and let the xp pool depth (bufs) pace DMA issue by
    # PE progress so at most ~5 chunks are ever in flight.
    X_SCHED = [1, 1, 1, 1, 2, 2] + [4] * 9 + [2, 2, 1, 1]
    W_SCHED = [4, 12, 34]
    assert sum(X_SCHED) == KT and sum(W_SCHED) == KT
    with TileContext(nc) as tc:
        with (
            tc.tile_pool(name="wp", bufs=1) as wp,
            tc.tile_pool(name="xp", bufs=5) as xp,
            tc.tile_pool(name="op", bufs=1) as op,
            tc.tile_pool(name="ps", bufs=1, space="PSUM") as ps,
        ):
            # HAM warm-up: the PE clock-gate holds matmuls at 1.2 GHz until
            # ~3.4us of sustained activity.  Burn the DMA-fill window (no real
            # operands on chip yet) on dummy matmuls so real MMs start at
            # 2.4 GHz and the PE is never idle long enough to re-throttle.
            # memset on DVE so the gpsimd queue starts the W DMAs immediately.
            dm = wp.tile([128, 512], fin, tag="warm", name="warmup")
            nc.vector.memset(dm[:], 0.0)
            ps_warm = ps.tile([128, 512], f32, tag="warm_ps", name="ps_warm")
            for _ in range(WARMUP_MMS):
                nc.tensor.matmul(ps_warm[:], dm[:, 0:128], dm[:], start=True, stop=True)

            # W loads on the gpsimd SWDGE ring, leaving both HWDGE rings
            # (sync + scalar) free to alternate x chunks.
            # w_of[kt] -> (tile, col offset of that k-tile's weights)
            w_of = {}
            kt0 = 0
            for wc, n in enumerate(W_SCHED):
                wt = wp.tile([128, n * N_OUT], fin, tag=f"w{wc}", name=f"w{wc}")
                nc.gpsimd.dma_start(wt[:], w[:, kt0 * N_OUT:(kt0 + n) * N_OUT])
                for j in range(n):
                    w_of[kt0 + j] = (wt, j * N_OUT)
                kt0 += n

            # Persistent accumulators: 4 mel banks + 2 shared cb banks.  Each
            # cb bank holds two t-blocks' [40, TB] outputs col-tiled into
            # partitions 0:40 and 64:104 (concurrent matmuls via tile_position).
            psm = [ps.tile([128, TB], f32, tag=f"m{t}", name=f"psm{t}") for t in range(NTB)]
            psc = [ps.tile([128, TB], f32, tag=f"c{p}", name=f"psc{p}") for p in range(NTB // 2)]

            xc0 = 0
            for xc, xn in enumerate(X_SCHED):
                x_tile = xp.tile([128, xn * T], fin, tag="x", name="x_tile")
                ring = nc.sync if xc % 2 == 0 else nc.scalar
                if xn == 1:
                    ring.dma_start(x_tile[:], xt[xc0 * 128:(xc0 + 1) * 128, :])
                else:
                    ring.dma_start(
                        x_tile[:].rearrange("p (a t) -> p a t", a=xn),
                        xt[xc0 * 128:(xc0 + xn) * 128, :].rearrange(
                            "(a p) t -> p a t", p=128
                        ),
                    )
                for a in range(xn):
                    kt = xc0 + a
                    wt, j = w_of[kt]
                    lhs_m = wt[:, j: j + 128]
                    lhs_c = wt[:, j + 128: j + N_OUT]
                    first, last = kt == 0, kt == KT - 1

                    def rhs_of(t):
                        return x_tile[:, a * T + t * TB: a * T + (t + 1) * TB]

                    def cb_pair(p):
                        # two concurrent 40-col matmuls in distinct col groups
                        nc.tensor.matmul(psc[p][0:40, :], lhs_c, rhs_of(2 * p),
                                         start=first, stop=last, tile_position=(0, 0))
                        nc.tensor.matmul(psc[p][64:104, :], lhs_c, rhs_of(2 * p + 1),
                                         start=first, stop=last, tile_position=(0, 64))

                    if not last:
                        # group by stationary operand: 4 mel MMs, then cb pairs
                        for t in range(NTB):
                            nc.tensor.matmul(psm[t][:], lhs_m, rhs_of(t), start=first, stop=last)
                        cb_pair(0)
                        cb_pair(1)
                    else:
                        # final k-tile: finish banks in eviction order so PSUM
                        # evictions and output DMAs overlap the remaining MMs;
                        # end on the cheap-to-evict cb pair.
                        nc.tensor.matmul(psm[0][:], lhs_m, rhs_of(0), start=first, stop=last)
                        nc.tensor.matmul(psm[1][:], lhs_m, rhs_of(1), start=first, stop=last)
                        o_m01 = op.tile([128, 2 * TB], f16, tag="om01", name="o_m01")
                        nc.vector.tensor_copy(o_m01[:, 0:TB], psm[0][:])
                        nc.vector.tensor_copy(o_m01[:, TB:2 * TB], psm[1][:])
                        nc.sync.dma_start(out[0:128, 0:2 * TB], o_m01[:])

                        cb_pair(0)
                        o_c0 = op.tile([104, TB], f16, tag="oc0", name="o_c0")
                        nc.scalar.copy(o_c0[:], psc[0][0:104, :])
                        nc.scalar.dma_start(out[128:N_OUT, 0:TB], o_c0[0:40, :])
                        nc.scalar.dma_start(out[128:N_OUT, TB:2 * TB], o_c0[64:104, :])

                        nc.tensor.matmul(psm[2][:], lhs_m, rhs_of(2), start=first, stop=last)
                        nc.tensor.matmul(psm[3][:], lhs_m, rhs_of(3), start=first, stop=last)
                        o_m23 = op.tile([128, 2 * TB], f16, tag="om23", name="o_m23")
                        nc.vector.tensor_copy(o_m23[:, 0:TB], psm[2][:])
                        nc.vector.tensor_copy(o_m23[:, TB:2 * TB], psm[3][:])
                        nc.sync.dma_start(out[0:128, 2 * TB:T], o_m23[:])

                        cb_pair(1)
                        o_c1 = op.tile([104, TB], f16, tag="oc1", name="o_c1")
                        nc.scalar.copy(o_c1[:], psc[1][0:104, :])
                        nc.scalar.dma_start(out[128:N_OUT, 2 * TB:3 * TB], o_c1[0:40, :])
                        nc.scalar.dma_start(out[128:N_OUT, 3 * TB:T], o_c1[64:104, :])
                xc0 += xn
    return nc


def _get_nc():
    global _NC
    if _NC is None:
        _NC = _build_nc()
        if not _NC.is_finalized():
            _NC.finalize()
    return _NC


def kernel(**inputs):
    global LAST_RESULT
    melody = np.ascontiguousarray(np.asarray(inputs["melody_tensor"], dtype=np.float32))
    lyrics = np.ascontiguousarray(np.asarray(inputs["lyrics_tensor"], dtype=np.float32))
    emb = np.asarray(inputs["emb"], dtype=np.float32)
    conv_w = np.asarray(inputs["conv_w"], dtype=np.float32)
    conv_b = np.asarray(inputs["conv_b"], dtype=np.float32)
    w_chord = np.asarray(inputs["w_chord"], dtype=np.float32)
    w_beat = np.asarray(inputs["w_beat"], dtype=np.float32)
    w_mel = np.asarray(inputs["w_mel"], dtype=np.float32)
    b_heads = np.concatenate([
        np.asarray(inputs["b_chord"], dtype=np.float32),
        np.asarray(inputs["b_beat"], dtype=np.float32),
        np.asarray(inputs["b_mel"], dtype=np.float32),
    ])
    genre = int(np.asarray(inputs["genre"]).reshape(-1)[0])
    tempo = int(np.asarray(inputs["tempo"]).reshape(-1)[0])
    key_sig = int(np.asarray(inputs["key_sig"]).reshape(-1)[0])

    # Fold conv into head weights: W'[e] = k0*W[e+1] + k1*W[e] + k2*W[e-1]
    W = np.concatenate([w_chord, w_beat, w_mel], axis=1)  # [50937, 168]
    k0, k1, k2 = (float(v) for v in conv_w[0, 0, 1, :])
    Wp = k1 * W
    Wp[:-1] += k0 * W[1:]
    Wp[1:] += k2 * W[:-1]

    # Bias: head biases + conv bias * colsum(W) + context-embedding term
    ids = [genre, 10 + tempo, 20 + key_sig, 34]
    ctx = emb[ids].sum(axis=0).astype(np.float64)  # [256]
    bias = (
        b_heads.astype(np.float64)
        + float(conv_b[0]) * W.sum(axis=0, dtype=np.float64)
        + ctx @ Wp[0:256].astype(np.float64)
    )  # [168]

    # Device operands: xT [51200, 2048] (zero padded), W' rows 256.. packed
    np_dt = _np_in_dt()
    K_PAD = N_CORES * K_PER
    XT = np.zeros((K_PAD, T), np_dt)
    XT[0:256] = melody.T
    XT[256:K_GEMM] = lyrics.T
    Wg = np.zeros((K_PAD, N_OUT), np_dt)
    Wg[0:K_GEMM] = Wp[256:]

    in_maps = []
    for c in range(N_CORES):
        wc = (
            Wg[c * K_PER:(c + 1) * K_PER]
            .reshape(KT, 128, N_OUT)
            .transpose(1, 0, 2)
            .reshape(128, KT * N_OUT)
        )
        in_maps.append({
            "xt": XT[c * K_PER:(c + 1) * K_PER],
            "w": np.ascontiguousarray(wc),
        })

    trace = bool(os.environ.get("HARMONY_TRACE"))
    res = run_bass_kernel_spmd(_get_nc(), in_maps, core_ids=list(range(N_CORES)), trace=trace)
    LAST_RESULT = res

    acc = np.zeros((N_OUT, T), np.float64)
    for r in res.results:
        acc += r["out"].astype(np.float64)
    out = (acc + bias[:, None]).T
    return np.ascontiguousarray(out.astype(np.float32))


# revision 11
# speedup vs baseline: 1.0526x; 1.0420x over previous
"""HarmonyGenerator Trainium2 kernel.

Math: the reference's 3x3 conv on [T,1,1,D] degenerates to a 3-tap conv along
the feature axis (only the kernel's middle row touches data).  Conv and the
three linear heads are both linear, so the conv folds into the head weights
(W' = 3-tap correlation of W along K) and the constant context-embedding rows
plus conv bias fold into the output bias.  The device work is one GEMM:

    out[2048, 168] = [melody | lyrics][2048, 50681] @ W'[50681, 168] + bias

Sharding: K (feature) axis split 8 ways, 6400 rows per core (zero padded).
Each core reads 1/8 of x AND 1/8 of W (~28 MB fp16 -> ~69us memory floor at
the ~410 GB/s measured per-core HBM rate) and produces a partial [168, 2048];
partials are summed on the host during the gather/unshard step.

Device schedule per core (PE-rate ~= DMA-rate, "ridge"):
  per k-tile (lhsT = W tile [128k, 168], moving span 512 = ISA max):
    mm psm[t][128,512] += W[:,0:128]T @ xT[:, t*512:(t+1)*512]  (4 t-blocks)
    cb pairs: two concurrent 40-col matmuls per pass via disjoint column
    groups (tile_position (0,0) and (0,64)) -> 2 passes for all 4 t-blocks
  = 6 PE passes of 512 cycles per k-tile (~1.5us warm).

Warmup matmuls keep the PE HAM clock-gate busy (K=8/8, 2.4 GHz) while the
first DMA chunks land; the x stream (2 HWDGE rings, up to 2 MB chunks) then
feeds the PE slightly faster than it consumes, so HAM never re-throttles.
Output is evicted PSUM->SBUF as fp16 (DVE + ACT in parallel) and stored as
three DMAs split across both rings; the host upcasts and adds the bias.
"""

import os
import numpy as np

import concourse.bacc as bacc
import concourse.mybir as mybir
from concourse.tile import TileContext
from concourse.bass_utils import run_bass_kernel_spmd

# Problem shapes (hardcoded per contract)
T = 2048               # steps = length * 128
D_IN = 50937           # 256 ctx + 256 melody/vel + 50425 lyrics
K_GEMM = 50681         # melody(256) + lyrics(50425) features in the GEMM
N_OUT = 168            # 24 chord + 16 beat + 128 mel
N_CORES = 8
K_PER = 6400           # per-core K (8*6400 = 51200 >= 50681, zero padded)
KT = K_PER // 128      # 50 k-tiles per core
TB = 512               # t-block / moving span (ISA max matmul elements)
NTB = T // TB          # 4

_NC = None
LAST_RESULT = None     # BassKernelResults of the most recent run (for test.py)

DTYPE = os.environ.get("HARMONY_DTYPE", "fp16")
WARMUP_MMS = int(os.environ.get("HARMONY_WARMUP", "12"))


def _in_dt():
    return mybir.dt.float16 if DTYPE == "fp16" else mybir.dt.float32r


def _np_in_dt():
    return np.float16 if DTYPE == "fp16" else np.float32


def _build_nc():
    f32 = mybir.dt.float32
    f16 = mybir.dt.float16
    fin = _in_dt()
    nc = bacc.Bacc()
    xt = nc.dram_tensor("xt", [K_PER, T], fin, kind="ExternalInput")
    w = nc.dram_tensor("w", [128, KT * N_OUT], fin, kind="ExternalInput")
    out = nc.dram_tensor("out", [N_OUT, T], f16, kind="ExternalOutput")

    # The SDMA engines round-robin between ACTIVE QUEUES at packet
    # granularity but drain each queue strictly FIFO.  Outstanding
    # transfers spread across queues therefore complete nearly together
    # (the in-order frontier crawls), while a single queue completes them
    # in issue order at full rate.  So: put W and x on ONE ring (sync),
    # interleaved in exact k-tile consumption order; the scalar ring only
    # carries the output stores at the end, and gpsimd/SWDGE stays idle.
    X_SCHED = [1, 1] + [2] * 24
    W_SCHED = [4, 12, 12, 22]
    # x-chunk index after which each W chunk is enqueued (W chunk 0 goes
    # before everything; chunk w+1 must land before its first k-tile).
    W_AFTER_X = {1: 1, 2: 6, 3: 11}
    assert sum(X_SCHED) == KT and sum(W_SCHED) == KT
    with TileContext(nc) as tc:
        with (
            tc.tile_pool(name="wp", bufs=1) as wp,
            tc.tile_pool(name="xp", bufs=5) as xp,
            tc.tile_pool(name="op", bufs=1) as op,
            tc.tile_pool(name="ps", bufs=1, space="PSUM") as ps,
        ):
            # HAM warm-up: the PE clock-gate holds matmuls at 1.2 GHz until
            # ~3.4us of sustained activity.  Burn the DMA-fill window (no real
            # operands on chip yet) on dummy matmuls so real MMs start at
            # 2.4 GHz and the PE is never idle long enough to re-throttle.
            # memset on DVE so the gpsimd queue starts the W DMAs immediately.
            dm = wp.tile([128, 512], fin, tag="warm", name="warmup")
            nc.vector.memset(dm[:], 0.0)
            ps_warm = ps.tile([128, 512], f32, tag="warm_ps", name="ps_warm")
            for _ in range(WARMUP_MMS):
                nc.tensor.matmul(ps_warm[:], dm[:, 0:128], dm[:], start=True, stop=True)

            # w_of[kt] -> (tile, col offset of that k-tile's weights)
            w_tiles = []
            w_of = {}
            kt0 = 0
            for wc, n in enumerate(W_SCHED):
                wt = wp.tile([128, n * N_OUT], fin, tag=f"w{wc}", name=f"w{wc}")
                w_tiles.append((wt, kt0, n))
                for j in range(n):
                    w_of[kt0 + j] = (wt, j * N_OUT)
                kt0 += n

            def load_w(wc):
                wt, kt0, n = w_tiles[wc]
                nc.sync.dma_start(wt[:], w[:, kt0 * N_OUT:(kt0 + n) * N_OUT])

            load_w(0)

            # Persistent accumulators: 4 mel banks + 2 shared cb banks.  Each
            # cb bank holds two t-blocks' [40, TB] outputs col-tiled into
            # partitions 0:40 and 64:104 (concurrent matmuls via tile_position).
            psm = [ps.tile([128, TB], f32, tag=f"m{t}", name=f"psm{t}") for t in range(NTB)]
            psc = [ps.tile([128, TB], f32, tag=f"c{p}", name=f"psc{p}") for p in range(NTB // 2)]

            xc0 = 0
            for xc, xn in enumerate(X_SCHED):
                x_tile = xp.tile([128, xn * T], fin, tag="x", name="x_tile")
                if xn == 1:
                    nc.sync.dma_start(x_tile[:], xt[xc0 * 128:(xc0 + 1) * 128, :])
                else:
                    nc.sync.dma_start(
                        x_tile[:].rearrange("p (a t) -> p a t", a=xn),
                        xt[xc0 * 128:(xc0 + xn) * 128, :].rearrange(
                            "(a p) t -> p a t", p=128
                        ),
                    )
                for wc, after in W_AFTER_X.items():
                    if after == xc:
                        load_w(wc)
                for a in range(xn):
                    kt = xc0 + a
                    wt, j = w_of[kt]
                    lhs_m = wt[:, j: j + 128]
                    lhs_c = wt[:, j + 128: j + N_OUT]
                    first, last = kt == 0, kt == KT - 1

                    def rhs_of(t):
                        return x_tile[:, a * T + t * TB: a * T + (t + 1) * TB]

                    def cb_pair(p):
                        # two concurrent 40-col matmuls in distinct col groups
                        nc.tensor.matmul(psc[p][0:40, :], lhs_c, rhs_of(2 * p),
                                         start=first, stop=last, tile_position=(0, 0))
                        nc.tensor.matmul(psc[p][64:104, :], lhs_c, rhs_of(2 * p + 1),
                                         start=first, stop=last, tile_position=(0, 64))

                    if not last:
                        # group by stationary operand: 4 mel MMs, then cb pairs
                        for t in range(NTB):
                            nc.tensor.matmul(psm[t][:], lhs_m, rhs_of(t), start=first, stop=last)
                        cb_pair(0)
                        cb_pair(1)
                    else:
                        # final k-tile: finish banks in eviction order so PSUM
                        # evictions and output DMAs overlap the remaining MMs;
                        # end on the cheap-to-evict cb pair.
                        nc.tensor.matmul(psm[0][:], lhs_m, rhs_of(0), start=first, stop=last)
                        nc.tensor.matmul(psm[1][:], lhs_m, rhs_of(1), start=first, stop=last)
                        o_m01 = op.tile([128, 2 * TB], f16, tag="om01", name="o_m01")
                        nc.vector.tensor_copy(o_m01[:, 0:TB], psm[0][:])
                        nc.vector.tensor_copy(o_m01[:, TB:2 * TB], psm[1][:])
                        nc.scalar.dma_start(out[0:128, 0:2 * TB], o_m01[:])

                        cb_pair(0)
                        o_c0 = op.tile([104, TB], f16, tag="oc0", name="o_c0")
                        nc.scalar.copy(o_c0[:], psc[0][0:104, :])
                        nc.scalar.dma_start(out[128:N_OUT, 0:TB], o_c0[0:40, :])
                        nc.scalar.dma_start(out[128:N_OUT, TB:2 * TB], o_c0[64:104, :])

                        nc.tensor.matmul(psm[2][:], lhs_m, rhs_of(2), start=first, stop=last)
                        nc.tensor.matmul(psm[3][:], lhs_m, rhs_of(3), start=first, stop=last)
                        o_m23 = op.tile([128, 2 * TB], f16, tag="om23", name="o_m23")
                        nc.vector.tensor_copy(o_m23[:, 0:TB], psm[2][:])
                        nc.vector.tensor_copy(o_m23[:, TB:2 * TB], psm[3][:])
                        nc.scalar.dma_start(out[0:128, 2 * TB:T], o_m23[:])

                        cb_pair(1)
                        o_c1 = op.tile([104, TB], f16, tag="oc1", name="o_c1")
                        nc.scalar.copy(o_c1[:], psc[1][0:104, :])
                        nc.scalar.dma_start(out[128:N_OUT, 2 * TB:3 * TB], o_c1[0:40, :])
                        nc.scalar.dma_start(out[128:N_OUT, 3 * TB:T], o_c1[64:104, :])
                xc0 += xn
    return nc


def _get_nc():
    global _NC
    if _NC is None:
        _NC = _build_nc()
        if not _NC.is_finalized():
            _NC.finalize()
    return _NC


def kernel(**inputs):
    global LAST_RESULT
    melody = np.ascontiguousarray(np.asarray(inputs["melody_tensor"], dtype=np.float32))
    lyrics = np.ascontiguousarray(np.asarray(inputs["lyrics_tensor"], dtype=np.float32))
    emb = np.asarray(inputs["emb"], dtype=np.float32)
    conv_w = np.asarray(inputs["conv_w"], dtype=np.float32)
    conv_b = np.asarray(inputs["conv_b"], dtype=np.float32)
    w_chord = np.asarray(inputs["w_chord"], dtype=np.float32)
    w_beat = np.asarray(inputs["w_beat"], dtype=np.float32)
    w_mel = np.asarray(inputs["w_mel"], dtype=np.float32)
    b_heads = np.concatenate([
        np.asarray(inputs["b_chord"], dtype=np.float32),
        np.asarray(inputs["b_beat"], dtype=np.float32),
        np.asarray(inputs["b_mel"], dtype=np.float32),
    ])
    genre = int(np.asarray(inputs["genre"]).reshape(-1)[0])
    tempo = int(np.asarray(inputs["tempo"]).reshape(-1)[0])
    key_sig = int(np.asarray(inputs["key_sig"]).reshape(-1)[0])

    # Fold conv into head weights: W'[e] = k0*W[e+1] + k1*W[e] + k2*W[e-1]
    W = np.concatenate([w_chord, w_beat, w_mel], axis=1)  # [50937, 168]
    k0, k1, k2 = (float(v) for v in conv_w[0, 0, 1, :])
    Wp = k1 * W
    Wp[:-1] += k0 * W[1:]
    Wp[1:] += k2 * W[:-1]

    # Bias: head biases + conv bias * colsum(W) + context-embedding term
    ids = [genre, 10 + tempo, 20 + key_sig, 34]
    ctx = emb[ids].sum(axis=0).astype(np.float64)  # [256]
    bias = (
        b_heads.astype(np.float64)
        + float(conv_b[0]) * W.sum(axis=0, dtype=np.float64)
        + ctx @ Wp[0:256].astype(np.float64)
    )  # [168]

    # Device operands: xT [51200, 2048] (zero padded), W' rows 256.. packed
    np_dt = _np_in_dt()
    K_PAD = N_CORES * K_PER
    XT = np.zeros((K_PAD, T), np_dt)
    XT[0:256] = melody.T
    XT[256:K_GEMM] = lyrics.T
    Wg = np.zeros((K_PAD, N_OUT), np_dt)
    Wg[0:K_GEMM] = Wp[256:]

    in_maps = []
    for c in range(N_CORES):
        wc = (
            Wg[c * K_PER:(c + 1) * K_PER]
            .reshape(KT, 128, N_OUT)
            .transpose(1, 0, 2)
            .reshape(128, KT * N_OUT)
        )
        in_maps.append({
            "xt": XT[c * K_PER:(c + 1) * K_PER],
            "w": np.ascontiguousarray(wc),
        })

    trace = bool(os.environ.get("HARMONY_TRACE"))
    res = run_bass_kernel_spmd(_get_nc(), in_maps, core_ids=list(range(N_CORES)), trace=trace)
    LAST_RESULT = res

    acc = np.zeros((N_OUT, T), np.float64)
    for r in res.results:
        acc += r["out"].astype(np.float64)
    out = (acc + bias[:, None]).T
    return np.ascontiguousarray(out.astype(np.float32))
